# revision 1
# baseline (speedup 1.0000x reference)
"""Trainium2 Bass kernel for a 2-layer GCN regressor (gnn_message_passing).

Computation (matches the reference):
  deg_out/deg_in -> norm = max(deg,1)^-0.5
  layer:  x = (h * norm_src) @ W ; agg = segment_sum(x[src], dst) ;
          h' = relu(agg * norm_dst + b)
  pool:   per-graph mean over nodes, then hg @ W3 + b3 -> [G, 1]

Sharding: nodes are partitioned contiguously across the 8 cores (dst
partitioning).  Each core projects its owned nodes, the projected features are
AllGathered so every core holds the full [N,128] table in HBM, then each core
aggregates its owned destination blocks by indirect-DMA-gathering the source
rows (bf16, 256B each) and reducing them with one-hot matmuls into PSUM
(aggT = X_edges.T @ onehot(dst)).  Aggregation output is produced feature-major
([feat x node]) which directly feeds the next projection as lhsT.  Pooling is
done per-node-dot (h2 @ W3) + one-hot matmul over graph windows, combined with
a tiny AllReduce.

All heavy matmul traffic is bf16 (one-hot values are exact); accumulation is
fp32 in PSUM.
"""

import numpy as np
import ml_dtypes

BF16 = ml_dtypes.bfloat16
NC = 8  # cores


# ----------------------------------------------------------------- host prep
def _prep(h, src, dst, graph_ids, num_graphs, W1, b1, W2, b2, W3, b3):
    h = np.asarray(h, dtype=np.float32)
    src = np.asarray(src, dtype=np.int64)
    dst = np.asarray(dst, dtype=np.int64)
    gid = np.asarray(graph_ids, dtype=np.int64)
    G = int(num_graphs)
    N, D = h.shape
    assert D == 128
    assert N % NC == 0
    NPC = N // NC               # real nodes per core
    B = (NPC + 127) // 128      # 128-node blocks per core
    NPAD = B * 128
    assert G % 128 == 0
    GC = G // 128               # output column groups

    owner = dst // NPC                      # edge -> owning core
    d_loc = dst - owner * NPC               # local node id on owner
    blk = d_loc // 128
    jloc = d_loc - blk * 128
    # padded global id of each source node
    s_owner = src // NPC
    s_pid = s_owner * NPAD + (src - s_owner * NPC)

    # per (core, block) edge counts -> shared tile grid
    counts = np.zeros((NC, B), dtype=np.int64)
    np.add.at(counts, (owner, blk), 1)
    T_list = np.maximum(1, (counts.max(axis=0) + 127) // 128).astype(np.int64)
    total_T = int(T_list.sum())
    off = np.zeros(B, dtype=np.int64)
    off[1:] = np.cumsum(T_list)[:-1]

    # CSR row pointers (degree metadata) over real node ids
    src_sorted = np.sort(src)
    dst_sorted = np.sort(dst)
    rp_src = np.searchsorted(src_sorted, np.arange(N + 1)).astype(np.int32)
    rp_dst = np.searchsorted(dst_sorted, np.arange(N + 1)).astype(np.int32)

    per_core = []
    order_all = np.lexsort((src, blk, owner))  # sort edges by (core, block, src)
    e_owner = owner[order_all]
    e_blk = blk[order_all]
    e_j = jloc[order_all]
    e_spid = s_pid[order_all]
    core_starts = np.searchsorted(e_owner, np.arange(NC + 1))

    for k in range(NC):
        lo, hi = core_starts[k], core_starts[k + 1]
        kb = e_blk[lo:hi]
        kj = e_j[lo:hi]
        ks = e_spid[lo:hi]
        blk_starts = np.searchsorted(kb, np.arange(B + 1))

        srcidx = np.zeros((128, total_T), dtype=np.int32)
        dstloc = np.full((128, total_T), -1.0, dtype=np.float32)
        for b in range(B):
            s, e = blk_starts[b], blk_starts[b + 1]
            cnt = e - s
            if cnt == 0:
                continue
            i = np.arange(cnt)
            col = off[b] + i // 128
            row = i % 128
            srcidx[row, col] = ks[s:e]
            dstloc[row, col] = kj[s:e].astype(np.float32)

        # graph one-hot over this core's graph window
        n0 = k * NPC
        kgid = gid[n0:n0 + NPC]
        g_base = int(kgid[0])
        span = int(kgid[-1]) - g_base + 1
        assert span <= 128, f"graph span {span} > 128 on core {k}"
        gone = np.zeros((128, NPAD), dtype=BF16)
        p_all = np.arange(NPC)
        gone[p_all % 128, (p_all // 128) * 128 + (kgid - g_base)] = BF16(1.0)
        gidx = (g_base + np.arange(128)).astype(np.int32)[:, None]  # pads land in dummy rows >= G

        # row-pointer tiles [128, B] (pad slots -> deg 0)
        node = n0 + np.arange(NPAD)
        valid = node < n0 + NPC
        nn = np.where(valid, node, n0)
        rsl = np.where(valid, rp_src[nn], 0).astype(np.int32).reshape(B, 128).T
        rsh = np.where(valid, rp_src[nn + 1], 0).astype(np.int32).reshape(B, 128).T
        rdl = np.where(valid, rp_dst[nn], 0).astype(np.int32).reshape(B, 128).T
        rdh = np.where(valid, rp_dst[nn + 1], 0).astype(np.int32).reshape(B, 128).T

        hT = np.zeros((128, NPAD), dtype=np.float32)
        hT[:, :NPC] = h[n0:n0 + NPC].T

        per_core.append(dict(
            hT=np.ascontiguousarray(hT),
            srcidx=srcidx, dstloc=dstloc, gonehot=gone, gidx=gidx,
            rsl=np.ascontiguousarray(rsl), rsh=np.ascontiguousarray(rsh),
            rdl=np.ascontiguousarray(rdl), rdh=np.ascontiguousarray(rdh),
        ))

    iota = np.broadcast_to(np.arange(128, dtype=np.float32), (128, 128))
    shared = dict(
        W1=np.asarray(W1, np.float32), W2=np.asarray(W2, np.float32),
        W3=np.asarray(W3, np.float32).reshape(128, 1),
        b1=np.asarray(b1, np.float32).reshape(128, 1),
        b2=np.asarray(b2, np.float32).reshape(128, 1),
        b3=np.broadcast_to(np.asarray(b3, np.float32).reshape(1, 1),
                           (128, 1)).astype(np.float32),
        iota=np.ascontiguousarray(iota.astype(BF16)),
    )
    in_maps = [{**shared, **pc} for pc in per_core]
    cfg = dict(N=N, NPC=NPC, B=B, NPAD=NPAD, G=G, GC=GC,
               T_list=[int(t) for t in T_list],
               off=[int(o) for o in off], total_T=total_T)
    return cfg, in_maps


# -------------------------------------------------------------- bass program
def _build(cfg):
    import concourse.bacc as bacc
    import concourse.mybir as mybir
    import concourse.tile as tile
    from concourse import bass
    from concourse.masks import make_identity

    dt = mybir.dt
    B = cfg["B"]
    NPAD = cfg["NPAD"]
    G = cfg["G"]
    GC = cfg["GC"]
    T_list = cfg["T_list"]
    off = cfg["off"]
    total_T = cfg["total_T"]
    Tmax = max(T_list)
    rg = [list(range(NC))]

    nc = bacc.Bacc("TRN2", target_bir_lowering=False, num_devices=NC)

    def din(name, shape, dtype):
        return nc.dram_tensor(name, shape, dtype, kind="ExternalInput")

    hT_in = din("hT", [128, NPAD], dt.float32)
    W1_in = din("W1", [128, 128], dt.float32)
    W2_in = din("W2", [128, 128], dt.float32)
    W3_in = din("W3", [128, 1], dt.float32)
    b1_in = din("b1", [128, 1], dt.float32)
    b2_in = din("b2", [128, 1], dt.float32)
    b3_in = din("b3", [128, 1], dt.float32)
    iota_in = din("iota", [128, 128], dt.bfloat16)
    srcidx_in = din("srcidx", [128, total_T], dt.int32)
    dstloc_in = din("dstloc", [128, total_T], dt.float32)
    gone_in = din("gonehot", [128, NPAD], dt.bfloat16)
    gidx_in = din("gidx", [128, 1], dt.int32)
    rsl_in = din("rsl", [128, B], dt.int32)
    rsh_in = din("rsh", [128, B], dt.int32)
    rdl_in = din("rdl", [128, B], dt.int32)
    rdh_in = din("rdh", [128, B], dt.int32)
    out_t = nc.dram_tensor("out", [G, 1], dt.float32, kind="ExternalOutput")

    x1_loc = nc.dram_tensor("x1_loc", [NPAD, 128], dt.bfloat16)
    x2_loc = nc.dram_tensor("x2_loc", [NPAD, 128], dt.bfloat16)
    x1_full = nc.dram_tensor("x1_full", [NPAD * NC, 128], dt.bfloat16,
                             addr_space="Shared")
    x2_full = nc.dram_tensor("x2_full", [NPAD * NC, 128], dt.bfloat16,
                             addr_space="Shared")
    pool_in = nc.dram_tensor("pool_in", [G + 128, 2], dt.float32)
    pool_out = nc.dram_tensor("pool_out", [G + 128, 2], dt.float32,
                              addr_space="Shared")

    with tile.TileContext(nc) as tc:
        with (
            tc.tile_pool(name="persist", bufs=1) as pp,
            tc.tile_pool(name="work", bufs=3) as wp,
            tc.tile_pool(name="gather", bufs=12) as gp,
            tc.tile_pool(name="onehot", bufs=4) as op_,
            tc.tile_pool(name="psA", bufs=2, space="PSUM") as psA,
            tc.tile_pool(name="psB", bufs=2, space="PSUM") as psB,
            tc.tile_pool(name="psC", bufs=1, space="PSUM") as psC,
            tc.tile_pool(name="psT", bufs=1, space="PSUM") as psT,
        ):
            # ---------- constants / weights ----------
            ident = pp.tile([128, 128], dt.float32)
            make_identity(nc, ident[:])
            iota = pp.tile([128, 128], dt.bfloat16)
            nc.sync.dma_start(out=iota[:], in_=iota_in[:])

            def load_w_bf(src_):
                f = wp.tile([128, 128], dt.float32, tag="wload")
                nc.sync.dma_start(out=f[:], in_=src_[:])
                bf = pp.tile([128, 128], dt.bfloat16, tag=src_.name + "bf")
                nc.vector.tensor_copy(out=bf[:], in_=f[:])
                return bf

            W1 = load_w_bf(W1_in)
            W2 = load_w_bf(W2_in)
            W3f = wp.tile([128, 1], dt.float32, tag="w3f")
            nc.sync.dma_start(out=W3f[:], in_=W3_in[:])
            W3 = pp.tile([128, 1], dt.bfloat16, tag="w3bf")
            nc.vector.tensor_copy(out=W3[:], in_=W3f[:])
            b1 = pp.tile([128, 1], dt.float32, tag="b1")
            nc.sync.dma_start(out=b1[:], in_=b1_in[:])
            b2 = pp.tile([128, 1], dt.float32, tag="b2")
            nc.sync.dma_start(out=b2[:], in_=b2_in[:])
            b3 = pp.tile([128, 1], dt.float32, tag="b3")
            nc.sync.dma_start(out=b3[:], in_=b3_in[:])

            # ---------- degree norms ----------
            def make_norm(lo_in, hi_in, tag):
                lo_i = wp.tile([128, B], dt.int32, tag=tag + "loi")
                hi_i = wp.tile([128, B], dt.int32, tag=tag + "hii")
                nc.sync.dma_start(out=lo_i[:], in_=lo_in[:])
                nc.sync.dma_start(out=hi_i[:], in_=hi_in[:])
                lo_f = wp.tile([128, B], dt.float32, tag=tag + "lof")
                hi_f = wp.tile([128, B], dt.float32, tag=tag + "hif")
                nc.vector.tensor_copy(out=lo_f[:], in_=lo_i[:])
                nc.vector.tensor_copy(out=hi_f[:], in_=hi_i[:])
                deg = wp.tile([128, B], dt.float32, tag=tag + "deg")
                nc.vector.tensor_tensor(out=deg[:], in0=hi_f[:], in1=lo_f[:],
                                        op=mybir.AluOpType.subtract)
                nc.vector.tensor_scalar_max(out=deg[:], in0=deg[:], scalar1=1.0)
                rec = wp.tile([128, B], dt.float32, tag=tag + "rec")
                nc.vector.reciprocal(out=rec[:], in_=deg[:])
                nrm = pp.tile([128, B], dt.float32, tag=tag + "nrm")
                nc.scalar.sqrt(out=nrm[:], in_=rec[:])
                return nrm

            norm_src = make_norm(rsl_in, rsh_in, "ns")
            norm_dst = make_norm(rdl_in, rdh_in, "nd")

            # broadcast norm_dst along partitions: nd_bc[:, b*128+j] = norm_dst[j, b]
            nd_bc = pp.tile([128, NPAD], dt.float32, tag="ndbc")
            for b in range(B):
                tp = psT.tile([128, 128], dt.float32, tag="ndtp")
                nc.tensor.transpose(
                    out=tp[:],
                    in_=norm_dst[:, b:b + 1].to_broadcast([128, 128]),
                    identity=ident[:],
                )
                nc.vector.tensor_copy(out=nd_bc[:, b * 128:(b + 1) * 128],
                                      in_=tp[:])

            # ---------- edge / pooling metadata ----------
            srcidx = pp.tile([128, total_T], dt.int32, tag="srcidx")
            nc.sync.dma_start(out=srcidx[:], in_=srcidx_in[:])
            dstloc = pp.tile([128, total_T], dt.float32, tag="dstloc")
            nc.sync.dma_start(out=dstloc[:], in_=dstloc_in[:])
            gone = pp.tile([128, NPAD], dt.bfloat16, tag="gone")
            nc.sync.dma_start(out=gone[:], in_=gone_in[:])
            gidx = pp.tile([128, 1], dt.int32, tag="gidx")
            nc.sync.dma_start(out=gidx[:], in_=gidx_in[:])

            h1T = pp.tile([128, NPAD], dt.bfloat16, tag="h1T")

            # ---------- phase 1: project layer-1 for owned nodes ----------
            CHUNK = 8
            for c0 in range(0, B, CHUNK):
                nb = min(CHUNK, B - c0)
                hf = wp.tile([128, CHUNK * 128], dt.float32, tag="hf")
                nc.sync.dma_start(out=hf[:, :nb * 128],
                                  in_=hT_in[:, c0 * 128:(c0 + nb) * 128])
                hb = wp.tile([128, CHUNK * 128], dt.bfloat16, tag="hb")
                nc.vector.tensor_copy(out=hb[:, :nb * 128], in_=hf[:, :nb * 128])
                for i in range(nb):
                    b = c0 + i
                    ps = psA.tile([128, 128], dt.float32, tag="proj")
                    nc.tensor.matmul(out=ps[:], lhsT=hb[:, i * 128:(i + 1) * 128],
                                     rhs=W1[:], start=True, stop=True)
                    xsb = wp.tile([128, 128], dt.bfloat16, tag="xsb")
                    nc.vector.tensor_scalar(out=xsb[:], in0=ps[:],
                                            scalar1=norm_src[:, b:b + 1],
                                            scalar2=None,
                                            op0=mybir.AluOpType.mult)
                    nc.sync.dma_start(out=x1_loc[b * 128:(b + 1) * 128, :],
                                      in_=xsb[:])

            # ---------- all-gather x1 ----------
            nc.gpsimd.collective_compute(
                "AllGather", mybir.AluOpType.bypass, replica_groups=rg,
                ins=[x1_loc[:].opt()], outs=[x1_full[:].opt()])

            # ---------- aggregation layer (shared for both layers) ----------
            def aggregate(b, x_full):
                T = T_list[b]
                o = off[b]
                agg = psB.tile([128, 128], dt.float32, tag="agg")
                for t in range(T):
                    xg = gp.tile([128, 128], dt.bfloat16, tag="xg")
                    nc.gpsimd.indirect_dma_start(
                        out=xg[:], out_offset=None,
                        in_=x_full[:],
                        in_offset=bass.IndirectOffsetOnAxis(
                            ap=srcidx[:, o + t:o + t + 1], axis=0),
                    )
                    S = op_.tile([128, 128], dt.bfloat16, tag="S")
                    nc.vector.tensor_scalar(out=S[:], in0=iota[:],
                                            scalar1=dstloc[:, o + t:o + t + 1],
                                            scalar2=None,
                                            op0=mybir.AluOpType.is_equal)
                    nc.tensor.matmul(out=agg[:], lhsT=xg[:],
                                     rhs=S[:], start=(t == 0),
                                     stop=(t == T - 1))
                return agg

            def finish_h(b, agg, bias, out_ap):
                t1 = wp.tile([128, 128], dt.float32, tag="t1")
                nc.vector.tensor_tensor(out=t1[:], in0=agg[:],
                                        in1=nd_bc[:, b * 128:(b + 1) * 128],
                                        op=mybir.AluOpType.mult)
                nc.scalar.activation(out=out_ap, in_=t1[:],
                                     func=mybir.ActivationFunctionType.Relu,
                                     bias=bias[:, 0:1], scale=1.0)

            # ---------- layer 1 aggregate + layer 2 project ----------
            for b in range(B):
                agg = aggregate(b, x1_full)
                finish_h(b, agg, b1, h1T[:, b * 128:(b + 1) * 128])
                ps2 = psA.tile([128, 128], dt.float32, tag="proj")
                nc.tensor.matmul(out=ps2[:], lhsT=h1T[:, b * 128:(b + 1) * 128],
                                 rhs=W2[:], start=True, stop=True)
                x2sb = wp.tile([128, 128], dt.bfloat16, tag="xsb")
                nc.vector.tensor_scalar(out=x2sb[:], in0=ps2[:],
                                        scalar1=norm_src[:, b:b + 1],
                                        scalar2=None, op0=mybir.AluOpType.mult)
                nc.sync.dma_start(out=x2_loc[b * 128:(b + 1) * 128, :],
                                  in_=x2sb[:])

            # ---------- all-gather x2 ----------
            nc.gpsimd.collective_compute(
                "AllGather", mybir.AluOpType.bypass, replica_groups=rg,
                ins=[x2_loc[:].opt()], outs=[x2_full[:].opt()])

            # ---------- layer 2 aggregate + pooling ----------
            pool_acc = pp.tile([128, 2], dt.float32, tag="poolacc")
            nc.gpsimd.memset(pool_acc[:], 0.0)
            ones_col = pp.tile([128, 1], dt.bfloat16, tag="ones")
            nc.gpsimd.memset(ones_col[:], 1.0)

            for b in range(B):
                agg = aggregate(b, x2_full)
                h2 = wp.tile([128, 128], dt.bfloat16, tag="h2")
                finish_h(b, agg, b2, h2[:])
                psd = psC.tile([128, 1], dt.float32, tag="dots")
                nc.tensor.matmul(out=psd[:], lhsT=h2[:], rhs=W3[:],
                                 start=True, stop=True)
                rhs2 = wp.tile([128, 2], dt.bfloat16, tag="rhs2")
                nc.vector.tensor_copy(out=rhs2[:, 0:1], in_=psd[:])
                nc.vector.tensor_copy(out=rhs2[:, 1:2], in_=ones_col[:])
                psp = psC.tile([128, 2], dt.float32, tag="poolmm")
                nc.tensor.matmul(out=psp[:],
                                 lhsT=gone[:, b * 128:(b + 1) * 128],
                                 rhs=rhs2[:], start=True, stop=True)
                nc.vector.tensor_add(out=pool_acc[:], in0=pool_acc[:],
                                     in1=psp[:])

            # ---------- combine pools across cores ----------
            zt = wp.tile([128, 2 * (GC + 1)], dt.float32, tag="zt")
            nc.gpsimd.memset(zt[:], 0.0)
            nc.sync.dma_start(
                out=bass.AP(pool_in, 0, [[2, 128], [256, GC + 1], [1, 2]]),
                in_=zt[:].rearrange("p (g c) -> p g c", c=2))
            nc.gpsimd.indirect_dma_start(
                out=pool_in[:],
                out_offset=bass.IndirectOffsetOnAxis(ap=gidx[:, 0:1], axis=0),
                in_=pool_acc[:], in_offset=None)
            nc.gpsimd.collective_compute(
                "AllReduce", mybir.AluOpType.add, replica_groups=rg,
                ins=[pool_in[:].opt()], outs=[pool_out[:].opt()])

            # ---------- finish: out = dot/cnt + b3 ----------
            # element (g=128*j+p, c) of pool_out lands at [p, j] of two tiles
            dots_t = wp.tile([128, GC], dt.float32, tag="dotst")
            cnt = wp.tile([128, GC], dt.float32, tag="cnt")
            nc.sync.dma_start(
                out=dots_t[:], in_=bass.AP(pool_out, 0, [[2, 128], [256, GC]]))
            nc.sync.dma_start(
                out=cnt[:], in_=bass.AP(pool_out, 1, [[2, 128], [256, GC]]))
            nc.vector.tensor_scalar_max(out=cnt[:], in0=cnt[:], scalar1=1.0)
            rec = wp.tile([128, GC], dt.float32, tag="recc")
            nc.vector.reciprocal(out=rec[:], in_=cnt[:])
            res = wp.tile([128, GC], dt.float32, tag="res")
            nc.vector.tensor_tensor(out=res[:], in0=dots_t[:], in1=rec[:],
                                    op=mybir.AluOpType.mult)
            nc.vector.tensor_scalar(out=res[:], in0=res[:], scalar1=b3[:, 0:1],
                                    scalar2=None, op0=mybir.AluOpType.add)
            nc.sync.dma_start(
                out=bass.AP(out_t, 0, [[1, 128], [128, GC]]), in_=res[:])

    nc.compile()
    return nc


_CACHE = {}


def _get_nc(cfg):
    key = (cfg["N"], cfg["G"], tuple(cfg["T_list"]))
    if key not in _CACHE:
        _CACHE[key] = _build(cfg)
    return _CACHE[key]


def kernel(**inputs) -> np.ndarray:
    from concourse import bass_utils

    cfg, in_maps = _prep(**inputs)
    nc = _get_nc(cfg)
    res = bass_utils.run_bass_kernel_spmd(
        nc, in_maps, core_ids=list(range(NC)), trace=False)
    return np.asarray(res.results[0]["out"], dtype=np.float32)



# revision 5
# speedup vs baseline: 1.1840x; 1.1840x over previous
"""Trainium2 Bass kernel for a 2-layer GCN regressor (gnn_message_passing).

Computation (matches the reference):
  deg_out/deg_in -> norm = max(deg,1)^-0.5
  layer:  x = (h * norm_src) @ W ; agg = segment_sum(x[src], dst) ;
          h' = relu(agg * norm_dst + b)
  pool:   per-graph mean over nodes, then hg @ W3 + b3 -> [G, 1]

Sharding: nodes are partitioned contiguously across the 8 cores (dst
partitioning).  Each core projects its owned nodes, the projected features are
AllGathered so every core holds the full [N,128] table in HBM, then each core
aggregates its owned destination blocks by gathering the source rows (bf16,
256B each) and reducing them with one-hot matmuls into PSUM
(aggT = X_edges.T @ onehot(dst)).

The gather uses dma_gather (batched SWDGE gather, ~1us fixed cost per call
amortized over thousands of rows) instead of per-128-row indirect DMAs.
dma_gather indices are int16, so the row table is split into 4 base ranges of
32768 rows; edges are grouped by (dst block, src range) at prep time.  One-hot
tiles are built with a single 3D-broadcast is_equal per block.  Norms are
precomputed on the host.
"""

import numpy as np
import ml_dtypes

BF16 = ml_dtypes.bfloat16
NC = 8        # cores
RSZ = 32768   # dma_gather int16 index range per table slice
CH = 4        # dst blocks per gather chunk


# ----------------------------------------------------------------- host prep
def _prep(h, src, dst, graph_ids, num_graphs, W1, b1, W2, b2, W3, b3):
    h = np.asarray(h, dtype=np.float32)
    src = np.asarray(src, dtype=np.int64)
    dst = np.asarray(dst, dtype=np.int64)
    gid = np.asarray(graph_ids, dtype=np.int64)
    G = int(num_graphs)
    N, D = h.shape
    assert D == 128
    assert N % NC == 0
    NPC = N // NC               # real nodes per core
    B = (NPC + 127) // 128      # 128-node blocks per core
    NPAD = B * 128
    assert G % 128 == 0
    GC = G // 128               # output column groups
    RR = (NPAD * NC + RSZ - 1) // RSZ   # src ranges

    owner = dst // NPC
    d_loc = dst - owner * NPC
    blk = d_loc // 128
    jloc = d_loc - blk * 128
    s_owner = src // NPC
    s_pid = s_owner * NPAD + (src - s_owner * NPC)   # padded global src id
    s_rng = s_pid // RSZ

    # per (core, block, range) counts -> shared tile grid (max over cores)
    counts = np.zeros((NC, B, RR), dtype=np.int64)
    np.add.at(counts, (owner, blk, s_rng), 1)
    T_br = np.zeros((B, RR), dtype=np.int64)
    cmax = counts.max(axis=0)
    T_br = (cmax + 127) // 128          # 0 allowed when no core has edges
    empty = T_br.sum(axis=1) == 0
    T_br[empty, 0] = 1                  # keep >=1 tile so PSUM is initialized
    Tb = T_br.sum(axis=1)               # tiles per block
    total_cols = int(Tb.sum())
    boff = np.zeros(B, dtype=np.int64)
    boff[1:] = np.cumsum(Tb)[:-1]
    pre_r = np.cumsum(T_br, axis=1) - T_br   # [B, RR] within-block col offset

    # chunk layout: chunks of CH blocks, per chunk per range one gather call
    chunks = [list(range(c0, min(c0 + CH, B))) for c0 in range(0, B, CH)]
    # per call (c, r): width in gathered rows
    call_w = [[int(T_br[bs, r].sum()) * 128 for r in range(RR)] for bs in
              [np.array(c) for c in chunks]]
    W_chunk = [sum(ws) for ws in call_w]
    Wmax = max(W_chunk)
    # idx16 global column offsets per call
    icol = []
    ic = 0
    for ci in range(len(chunks)):
        row = []
        for r in range(RR):
            row.append(ic)
            ic += call_w[ci][r] // 16
        icol.append(row)
    ICtot = max(ic, 1)

    # degree norms (host)
    deg_out = np.bincount(src, minlength=N).astype(np.float32)
    deg_in = np.bincount(dst, minlength=N).astype(np.float32)
    ns_full = 1.0 / np.sqrt(np.maximum(deg_out, 1.0))
    nd_full = 1.0 / np.sqrt(np.maximum(deg_in, 1.0))

    order_all = np.lexsort((s_pid, s_rng, blk, owner))
    e_owner = owner[order_all]
    e_blk = blk[order_all]
    e_rng = s_rng[order_all]
    e_j = jloc[order_all]
    e_spid = s_pid[order_all]
    core_starts = np.searchsorted(e_owner, np.arange(NC + 1))

    per_core = []
    for k in range(NC):
        lo, hi = core_starts[k], core_starts[k + 1]
        kb = e_blk[lo:hi]
        kr = e_rng[lo:hi]
        kj = e_j[lo:hi]
        ks = e_spid[lo:hi]
        key = kb * RR + kr
        brs = np.searchsorted(key, np.arange(B * RR + 1))

        dstloc = np.full((128, total_cols), -1.0, dtype=np.float32)
        idx16 = np.zeros((128, ICtot), dtype=np.int16)
        for ci, cblocks in enumerate(chunks):
            for r in range(RR):
                if call_w[ci][r] == 0:
                    continue
                vals = []
                for b in cblocks:
                    s, e = brs[b * RR + r], brs[b * RR + r + 1]
                    cnt = e - s
                    w = int(T_br[b, r]) * 128
                    v = np.zeros(w, dtype=np.int16)
                    v[:cnt] = (ks[s:e] - r * RSZ).astype(np.int16)
                    vals.append(v)
                    # dstloc for this (b, r) span
                    if cnt:
                        i = np.arange(cnt)
                        col = boff[b] + pre_r[b, r] + i // 128
                        dstloc[i % 128, col] = kj[s:e].astype(np.float32)
                allv = np.concatenate(vals)
                c16 = allv.reshape(-1, 16).T          # [16, X]
                io = icol[ci][r]
                idx16[:, io:io + c16.shape[1]] = np.tile(c16, (8, 1))

        # graph one-hot over this core's graph window
        n0 = k * NPC
        kgid = gid[n0:n0 + NPC]
        g_base = int(kgid[0])
        span = int(kgid[-1]) - g_base + 1
        assert span <= 128, f"graph span {span} > 128 on core {k}"
        gone = np.zeros((128, NPAD), dtype=BF16)
        p_all = np.arange(NPC)
        gone[p_all % 128, (p_all // 128) * 128 + (kgid - g_base)] = BF16(1.0)
        gidx = (g_base + np.arange(128)).astype(np.int32)[:, None]

        ns_row = np.zeros(NPAD, dtype=np.float32)
        ns_row[:NPC] = ns_full[n0:n0 + NPC]
        ns_t = ns_row.reshape(B, 128).T
        nd_row = np.zeros(NPAD, dtype=np.float32)
        nd_row[:NPC] = nd_full[n0:n0 + NPC]
        nd_bc = np.broadcast_to(nd_row, (128, NPAD)).astype(BF16)

        hT = np.zeros((128, NPAD), dtype=BF16)
        hT[:, :NPC] = h[n0:n0 + NPC].T.astype(BF16)

        per_core.append(dict(
            hT=np.ascontiguousarray(hT),
            idx16=idx16, dstloc=dstloc, gonehot=gone, gidx=gidx,
            ns=np.ascontiguousarray(ns_t), ndbc=np.ascontiguousarray(nd_bc),
        ))

    iota = np.broadcast_to(np.arange(128, dtype=np.float32), (128, 128))
    shared = dict(
        W1=np.asarray(W1, np.float32), W2=np.asarray(W2, np.float32),
        W3=np.asarray(W3, np.float32).reshape(128, 1),
        b1=np.asarray(b1, np.float32).reshape(128, 1),
        b2=np.asarray(b2, np.float32).reshape(128, 1),
        b3=np.broadcast_to(np.asarray(b3, np.float32).reshape(1, 1),
                           (128, 1)).astype(np.float32),
        iota=np.ascontiguousarray(iota.astype(BF16)),
    )
    in_maps = [{**shared, **pc} for pc in per_core]
    cfg = dict(N=N, NPC=NPC, B=B, NPAD=NPAD, G=G, GC=GC, RR=RR,
               T_br=[[int(x) for x in row] for row in T_br],
               boff=[int(o) for o in boff],
               pre_r=[[int(x) for x in row] for row in pre_r],
               chunks=chunks, call_w=call_w, icol=icol,
               W_chunk=W_chunk, Wmax=Wmax, ICtot=ICtot,
               total_cols=total_cols, Tmax=int(Tb.max()))
    return cfg, in_maps


# -------------------------------------------------------------- bass program
def _build(cfg):
    import concourse.bacc as bacc
    import concourse.mybir as mybir
    import concourse.tile as tile
    from concourse import bass

    dt = mybir.dt
    B = cfg["B"]
    NPAD = cfg["NPAD"]
    G = cfg["G"]
    GC = cfg["GC"]
    RR = cfg["RR"]
    T_br = cfg["T_br"]
    boff = cfg["boff"]
    pre_r = cfg["pre_r"]
    chunks = cfg["chunks"]
    call_w = cfg["call_w"]
    icol = cfg["icol"]
    Wmax = cfg["Wmax"]
    ICtot = cfg["ICtot"]
    total_cols = cfg["total_cols"]
    Tmax = cfg["Tmax"]
    rg = [list(range(NC))]

    nc = bacc.Bacc("TRN2", target_bir_lowering=False, num_devices=NC)

    def din(name, shape, dtype):
        return nc.dram_tensor(name, shape, dtype, kind="ExternalInput")

    hT_in = din("hT", [128, NPAD], dt.bfloat16)
    W1_in = din("W1", [128, 128], dt.float32)
    W2_in = din("W2", [128, 128], dt.float32)
    W3_in = din("W3", [128, 1], dt.float32)
    b1_in = din("b1", [128, 1], dt.float32)
    b2_in = din("b2", [128, 1], dt.float32)
    b3_in = din("b3", [128, 1], dt.float32)
    iota_in = din("iota", [128, 128], dt.bfloat16)
    idx16_in = din("idx16", [128, ICtot], dt.int16)
    dstloc_in = din("dstloc", [128, total_cols], dt.float32)
    gone_in = din("gonehot", [128, NPAD], dt.bfloat16)
    gidx_in = din("gidx", [128, 1], dt.int32)
    ns_in = din("ns", [128, B], dt.float32)
    ndbc_in = din("ndbc", [128, NPAD], dt.bfloat16)
    out_t = nc.dram_tensor("out", [G, 1], dt.float32, kind="ExternalOutput")

    x1_loc = nc.dram_tensor("x1_loc", [NPAD, 128], dt.bfloat16)
    x2_loc = nc.dram_tensor("x2_loc", [NPAD, 128], dt.bfloat16)
    x1_full = nc.dram_tensor("x1_full", [NPAD * NC, 128], dt.bfloat16,
                             addr_space="Shared")
    x2_full = nc.dram_tensor("x2_full", [NPAD * NC, 128], dt.bfloat16,
                             addr_space="Shared")
    pool_in = nc.dram_tensor("pool_in", [G + 128, 2], dt.float32)
    pool_out = nc.dram_tensor("pool_out", [G + 128, 2], dt.float32,
                              addr_space="Shared")

    with tile.TileContext(nc) as tc:
        with (
            tc.tile_pool(name="persist", bufs=1) as pp,
            tc.tile_pool(name="work", bufs=3) as wp,
            tc.tile_pool(name="gather", bufs=2) as gp,
            tc.tile_pool(name="onehot", bufs=3) as op_,
            tc.tile_pool(name="psA", bufs=2, space="PSUM") as psA,
            tc.tile_pool(name="psB", bufs=2, space="PSUM") as psB,
            tc.tile_pool(name="psC", bufs=1, space="PSUM") as psC,
        ):
            # ---------- constants / weights / metadata ----------
            iota = pp.tile([128, 128], dt.bfloat16)
            nc.sync.dma_start(out=iota[:], in_=iota_in[:])

            def load_w_bf(src_):
                f = wp.tile([128, 128], dt.float32, tag="wload")
                nc.sync.dma_start(out=f[:], in_=src_[:])
                bf = pp.tile([128, 128], dt.bfloat16, tag=src_.name + "bf")
                nc.vector.tensor_copy(out=bf[:], in_=f[:])
                return bf

            W1 = load_w_bf(W1_in)
            W2 = load_w_bf(W2_in)
            W3f = wp.tile([128, 1], dt.float32, tag="w3f")
            nc.sync.dma_start(out=W3f[:], in_=W3_in[:])
            W3 = pp.tile([128, 1], dt.bfloat16, tag="w3bf")
            nc.vector.tensor_copy(out=W3[:], in_=W3f[:])
            b1 = pp.tile([128, 1], dt.float32, tag="b1")
            nc.sync.dma_start(out=b1[:], in_=b1_in[:])
            b2 = pp.tile([128, 1], dt.float32, tag="b2")
            nc.sync.dma_start(out=b2[:], in_=b2_in[:])
            b3 = pp.tile([128, 1], dt.float32, tag="b3")
            nc.sync.dma_start(out=b3[:], in_=b3_in[:])

            norm_src = pp.tile([128, B], dt.float32, tag="ns")
            nc.sync.dma_start(out=norm_src[:], in_=ns_in[:])
            nd_bc = pp.tile([128, NPAD], dt.bfloat16, tag="ndbc")
            nc.sync.dma_start(out=nd_bc[:], in_=ndbc_in[:])

            idx16 = pp.tile([128, ICtot], dt.int16, tag="idx16")
            nc.sync.dma_start(out=idx16[:], in_=idx16_in[:])
            dstloc = pp.tile([128, total_cols], dt.float32, tag="dstloc")
            nc.sync.dma_start(out=dstloc[:], in_=dstloc_in[:])
            gone = pp.tile([128, NPAD], dt.bfloat16, tag="gone")
            nc.sync.dma_start(out=gone[:], in_=gone_in[:])
            gidx = pp.tile([128, 1], dt.int32, tag="gidx")
            nc.sync.dma_start(out=gidx[:], in_=gidx_in[:])

            # ---------- phase 1: project layer-1 for owned nodes ----------
            PCH = 8
            for c0 in range(0, B, PCH):
                nb = min(PCH, B - c0)
                hb = wp.tile([128, PCH * 128], dt.bfloat16, tag="hb")
                nc.sync.dma_start(out=hb[:, :nb * 128],
                                  in_=hT_in[:, c0 * 128:(c0 + nb) * 128])
                for i in range(nb):
                    b = c0 + i
                    ps = psA.tile([128, 128], dt.float32, tag="proj")
                    nc.tensor.matmul(out=ps[:], lhsT=hb[:, i * 128:(i + 1) * 128],
                                     rhs=W1[:], start=True, stop=True)
                    xsb = wp.tile([128, 128], dt.bfloat16, tag="xsb")
                    nc.vector.tensor_scalar(out=xsb[:], in0=ps[:],
                                            scalar1=norm_src[:, b:b + 1],
                                            scalar2=None,
                                            op0=mybir.AluOpType.mult)
                    nc.sync.dma_start(out=x1_loc[b * 128:(b + 1) * 128, :],
                                      in_=xsb[:])

            # ---------- all-gather x1 ----------
            nc.gpsimd.collective_compute(
                "AllGather", mybir.AluOpType.bypass, replica_groups=rg,
                ins=[x1_loc[:].opt()], outs=[x1_full[:].opt()])

            # ---------- shared per-layer machinery ----------
            def gather_chunk(ci, x_full):
                cw = sum(call_w[ci])
                xgc = gp.tile([128, Wmax], dt.bfloat16, tag="xgc")
                go = 0
                for r in range(RR):
                    w = call_w[ci][r]
                    if w == 0:
                        continue
                    lo = r * RSZ
                    hi = min(lo + RSZ, NPAD * NC)
                    nc.gpsimd.dma_gather(
                        out_ap=xgc[:, go:go + w].rearrange(
                            "p (j e) -> p j e", e=128),
                        in_ap=x_full[lo:hi, :],
                        idxs_ap=idx16[:, icol[ci][r]:icol[ci][r] + w // 16],
                        num_idxs=w,
                        num_idxs_reg=w,
                        elem_size=128,
                        single_packet=False,
                    )
                    go += w
                return xgc

            def block_agg(ci, b, xgc):
                # xg column offset of (b, r) inside the chunk tile
                Tb = sum(T_br[b])
                S = op_.tile([128, Tmax * 128], dt.bfloat16, tag="S")
                nc.vector.tensor_tensor(
                    out=S[:, :Tb * 128].rearrange("p (t j) -> p t j", j=128),
                    in0=iota[:].unsqueeze(1).to_broadcast([128, Tb, 128]),
                    in1=dstloc[:, boff[b]:boff[b] + Tb].unsqueeze(2)
                        .to_broadcast([128, Tb, 128]),
                    op=mybir.AluOpType.is_equal,
                )
                agg = psB.tile([128, 128], dt.float32, tag="agg")
                nmm = 0
                go = 0
                for r in range(RR):
                    xoff = go + sum(T_br[bb][r] for bb in chunks[ci]
                                    if bb < b) * 128
                    for t in range(T_br[b][r]):
                        scol = (pre_r[b][r] + t) * 128
                        nc.tensor.matmul(
                            out=agg[:],
                            lhsT=xgc[:, xoff + t * 128:xoff + (t + 1) * 128],
                            rhs=S[:, scol:scol + 128],
                            start=(nmm == 0), stop=(nmm == Tb - 1))
                        nmm += 1
                    go += call_w[ci][r]
                return agg

            def finish_h(b, agg, bias, out_ap):
                t1 = wp.tile([128, 128], dt.float32, tag="t1")
                nc.vector.tensor_tensor(out=t1[:], in0=agg[:],
                                        in1=nd_bc[:, b * 128:(b + 1) * 128],
                                        op=mybir.AluOpType.mult)
                nc.scalar.activation(out=out_ap, in_=t1[:],
                                     func=mybir.ActivationFunctionType.Relu,
                                     bias=bias[:, 0:1], scale=1.0)

            # ---------- layer 1 aggregate + layer 2 project ----------
            for ci in range(len(chunks)):
                xgc = gather_chunk(ci, x1_full)
                for b in chunks[ci]:
                    agg = block_agg(ci, b, xgc)
                    h1 = wp.tile([128, 128], dt.bfloat16, tag="h1")
                    finish_h(b, agg, b1, h1[:])
                    ps2 = psA.tile([128, 128], dt.float32, tag="proj")
                    nc.tensor.matmul(out=ps2[:], lhsT=h1[:],
                                     rhs=W2[:], start=True, stop=True)
                    x2sb = wp.tile([128, 128], dt.bfloat16, tag="xsb")
                    nc.vector.tensor_scalar(out=x2sb[:], in0=ps2[:],
                                            scalar1=norm_src[:, b:b + 1],
                                            scalar2=None,
                                            op0=mybir.AluOpType.mult)
                    nc.sync.dma_start(out=x2_loc[b * 128:(b + 1) * 128, :],
                                      in_=x2sb[:])

            # ---------- all-gather x2 ----------
            nc.gpsimd.collective_compute(
                "AllGather", mybir.AluOpType.bypass, replica_groups=rg,
                ins=[x2_loc[:].opt()], outs=[x2_full[:].opt()])

            # ---------- layer 2 aggregate + pooling ----------
            pool_acc = pp.tile([128, 2], dt.float32, tag="poolacc")
            nc.gpsimd.memset(pool_acc[:], 0.0)
            ones_col = pp.tile([128, 1], dt.bfloat16, tag="ones")
            nc.gpsimd.memset(ones_col[:], 1.0)

            for ci in range(len(chunks)):
                xgc = gather_chunk(ci, x2_full)
                for b in chunks[ci]:
                    agg = block_agg(ci, b, xgc)
                    h2 = wp.tile([128, 128], dt.bfloat16, tag="h2")
                    finish_h(b, agg, b2, h2[:])
                    psd = psC.tile([128, 1], dt.float32, tag="dots")
                    nc.tensor.matmul(out=psd[:], lhsT=h2[:], rhs=W3[:],
                                     start=True, stop=True)
                    rhs2 = wp.tile([128, 2], dt.bfloat16, tag="rhs2")
                    nc.vector.tensor_copy(out=rhs2[:, 0:1], in_=psd[:])
                    nc.vector.tensor_copy(out=rhs2[:, 1:2], in_=ones_col[:])
                    psp = psC.tile([128, 2], dt.float32, tag="poolmm")
                    nc.tensor.matmul(out=psp[:],
                                     lhsT=gone[:, b * 128:(b + 1) * 128],
                                     rhs=rhs2[:], start=True, stop=True)
                    nc.vector.tensor_add(out=pool_acc[:], in0=pool_acc[:],
                                         in1=psp[:])

            # ---------- combine pools across cores ----------
            zt = wp.tile([128, 2 * (GC + 1)], dt.float32, tag="zt")
            nc.gpsimd.memset(zt[:], 0.0)
            nc.sync.dma_start(
                out=bass.AP(pool_in, 0, [[2, 128], [256, GC + 1], [1, 2]]),
                in_=zt[:].rearrange("p (g c) -> p g c", c=2))
            nc.gpsimd.indirect_dma_start(
                out=pool_in[:],
                out_offset=bass.IndirectOffsetOnAxis(ap=gidx[:, 0:1], axis=0),
                in_=pool_acc[:], in_offset=None)
            nc.gpsimd.collective_compute(
                "AllReduce", mybir.AluOpType.add, replica_groups=rg,
                ins=[pool_in[:].opt()], outs=[pool_out[:].opt()])

            # ---------- finish: out = dot/cnt + b3 ----------
            dots_t = wp.tile([128, GC], dt.float32, tag="dotst")
            cnt = wp.tile([128, GC], dt.float32, tag="cnt")
            nc.sync.dma_start(
                out=dots_t[:], in_=bass.AP(pool_out, 0, [[2, 128], [256, GC]]))
            nc.sync.dma_start(
                out=cnt[:], in_=bass.AP(pool_out, 1, [[2, 128], [256, GC]]))
            nc.vector.tensor_scalar_max(out=cnt[:], in0=cnt[:], scalar1=1.0)
            rec = wp.tile([128, GC], dt.float32, tag="recc")
            nc.vector.reciprocal(out=rec[:], in_=cnt[:])
            res = wp.tile([128, GC], dt.float32, tag="res")
            nc.vector.tensor_tensor(out=res[:], in0=dots_t[:], in1=rec[:],
                                    op=mybir.AluOpType.mult)
            nc.vector.tensor_scalar(out=res[:], in0=res[:], scalar1=b3[:, 0:1],
                                    scalar2=None, op0=mybir.AluOpType.add)
            nc.sync.dma_start(
                out=bass.AP(out_t, 0, [[1, 128], [128, GC]]), in_=res[:])

    nc.compile()
    return nc


_CACHE = {}


def _get_nc(cfg):
    key = (cfg["N"], cfg["G"], cfg["total_cols"], cfg["ICtot"])
    if key not in _CACHE:
        _CACHE[key] = _build(cfg)
    return _CACHE[key]


def kernel(**inputs) -> np.ndarray:
    from concourse import bass_utils

    cfg, in_maps = _prep(**inputs)
    nc = _get_nc(cfg)
    res = bass_utils.run_bass_kernel_spmd(
        nc, in_maps, core_ids=list(range(NC)), trace=False)
    return np.asarray(res.results[0]["out"], dtype=np.float32)


# revision 9
# speedup vs baseline: 1.1852x; 1.0010x over previous
"""Trainium2 Bass kernel for a 2-layer GCN regressor (gnn_message_passing).

Computation (matches the reference):
  deg_out/deg_in -> norm = max(deg,1)^-0.5
  layer:  x = (h * norm_src) @ W ; agg = segment_sum(x[src], dst) ;
          h' = relu(agg * norm_dst + b)
  pool:   per-graph mean over nodes, then hg @ W3 + b3 -> [G, 1]

Sharding: nodes are partitioned contiguously across the 8 cores (dst
partitioning).  Each core projects its owned nodes, the projected features are
AllGathered so every core holds the full [N,128] table in HBM, then each core
aggregates its owned destination blocks by gathering the source rows (bf16,
256B each) and reducing them with one-hot matmuls into PSUM
(aggT = X_edges.T @ onehot(dst)).

The gather uses dma_gather (batched SWDGE gather, ~1us fixed cost per call
amortized over thousands of rows) instead of per-128-row indirect DMAs.
dma_gather indices are int16, so the row table is split into 4 base ranges of
32768 rows; edges are grouped by (dst block, src range) at prep time.  One-hot
tiles are built with a single 3D-broadcast is_equal per block.  Norms are
precomputed on the host.
"""

import numpy as np
import ml_dtypes

BF16 = ml_dtypes.bfloat16
NC = 8        # cores
RSZ = 32768   # dma_gather int16 index range per table slice
CH = 4        # dst blocks per gather chunk


# ----------------------------------------------------------------- host prep
def _prep(h, src, dst, graph_ids, num_graphs, W1, b1, W2, b2, W3, b3):
    h = np.asarray(h, dtype=np.float32)
    src = np.asarray(src, dtype=np.int64)
    dst = np.asarray(dst, dtype=np.int64)
    gid = np.asarray(graph_ids, dtype=np.int64)
    G = int(num_graphs)
    N, D = h.shape
    assert D == 128
    assert N % NC == 0
    NPC = N // NC               # real nodes per core
    B = (NPC + 127) // 128      # 128-node blocks per core
    NPAD = B * 128
    assert G % 128 == 0
    GC = G // 128               # output column groups
    RR = (NPAD * NC + RSZ - 1) // RSZ   # src ranges

    owner = dst // NPC
    d_loc = dst - owner * NPC
    blk = d_loc // 128
    jloc = d_loc - blk * 128
    s_owner = src // NPC
    s_pid = s_owner * NPAD + (src - s_owner * NPC)   # padded global src id
    s_rng = s_pid // RSZ

    # per (core, block, range) counts -> shared tile grid (max over cores)
    counts = np.zeros((NC, B, RR), dtype=np.int64)
    np.add.at(counts, (owner, blk, s_rng), 1)
    T_br = np.zeros((B, RR), dtype=np.int64)
    cmax = counts.max(axis=0)
    T_br = (cmax + 127) // 128          # 0 allowed when no core has edges
    empty = T_br.sum(axis=1) == 0
    T_br[empty, 0] = 1                  # keep >=1 tile so PSUM is initialized
    Tb = T_br.sum(axis=1)               # tiles per block
    total_cols = int(Tb.sum())
    boff = np.zeros(B, dtype=np.int64)
    boff[1:] = np.cumsum(Tb)[:-1]
    pre_r = np.cumsum(T_br, axis=1) - T_br   # [B, RR] within-block col offset

    # chunk layout: chunks of CH blocks, per chunk per range one gather call
    chunks = [list(range(c0, min(c0 + CH, B))) for c0 in range(0, B, CH)]
    # per call (c, r): width in gathered rows
    call_w = [[int(T_br[bs, r].sum()) * 128 for r in range(RR)] for bs in
              [np.array(c) for c in chunks]]
    W_chunk = [sum(ws) for ws in call_w]
    Wmax = max(W_chunk)
    # idx16 global column offsets per call
    icol = []
    ic = 0
    for ci in range(len(chunks)):
        row = []
        for r in range(RR):
            row.append(ic)
            ic += call_w[ci][r] // 16
        icol.append(row)
    ICtot = max(ic, 1)

    # degree norms (host)
    deg_out = np.bincount(src, minlength=N).astype(np.float32)
    deg_in = np.bincount(dst, minlength=N).astype(np.float32)
    ns_full = 1.0 / np.sqrt(np.maximum(deg_out, 1.0))
    nd_full = 1.0 / np.sqrt(np.maximum(deg_in, 1.0))

    order_all = np.lexsort((s_pid, s_rng, blk, owner))
    e_owner = owner[order_all]
    e_blk = blk[order_all]
    e_rng = s_rng[order_all]
    e_j = jloc[order_all]
    e_spid = s_pid[order_all]
    core_starts = np.searchsorted(e_owner, np.arange(NC + 1))

    per_core = []
    for k in range(NC):
        lo, hi = core_starts[k], core_starts[k + 1]
        kb = e_blk[lo:hi]
        kr = e_rng[lo:hi]
        kj = e_j[lo:hi]
        ks = e_spid[lo:hi]
        key = kb * RR + kr
        brs = np.searchsorted(key, np.arange(B * RR + 1))

        dstloc = np.full((128, total_cols), -1.0, dtype=np.float32)
        idx16 = np.zeros((128, ICtot), dtype=np.int16)
        for ci, cblocks in enumerate(chunks):
            for r in range(RR):
                if call_w[ci][r] == 0:
                    continue
                vals = []
                for b in cblocks:
                    s, e = brs[b * RR + r], brs[b * RR + r + 1]
                    cnt = e - s
                    w = int(T_br[b, r]) * 128
                    v = np.zeros(w, dtype=np.int16)
                    v[:cnt] = (ks[s:e] - r * RSZ).astype(np.int16)
                    vals.append(v)
                    # dstloc for this (b, r) span
                    if cnt:
                        i = np.arange(cnt)
                        col = boff[b] + pre_r[b, r] + i // 128
                        dstloc[i % 128, col] = kj[s:e].astype(np.float32)
                allv = np.concatenate(vals)
                c16 = allv.reshape(-1, 16).T          # [16, X]
                io = icol[ci][r]
                idx16[:, io:io + c16.shape[1]] = np.tile(c16, (8, 1))

        # graph one-hot over this core's graph window
        n0 = k * NPC
        kgid = gid[n0:n0 + NPC]
        g_base = int(kgid[0])
        span = int(kgid[-1]) - g_base + 1
        assert span <= 128, f"graph span {span} > 128 on core {k}"
        gone = np.zeros((128, NPAD), dtype=BF16)
        p_all = np.arange(NPC)
        gone[p_all % 128, (p_all // 128) * 128 + (kgid - g_base)] = BF16(1.0)
        gidx = (g_base + np.arange(128)).astype(np.int32)[:, None]

        ns_row = np.zeros(NPAD, dtype=np.float32)
        ns_row[:NPC] = ns_full[n0:n0 + NPC]
        ns_t = ns_row.reshape(B, 128).T
        nd_row = np.zeros(NPAD, dtype=np.float32)
        nd_row[:NPC] = nd_full[n0:n0 + NPC]
        nd_bc = np.broadcast_to(nd_row, (128, NPAD)).astype(BF16)

        hT = np.zeros((128, NPAD), dtype=BF16)
        hT[:, :NPC] = h[n0:n0 + NPC].T.astype(BF16)

        per_core.append(dict(
            hT=np.ascontiguousarray(hT),
            idx16=idx16, dstloc=dstloc, gonehot=gone, gidx=gidx,
            ns=np.ascontiguousarray(ns_t), ndbc=np.ascontiguousarray(nd_bc),
        ))

    iota = np.broadcast_to(np.arange(128, dtype=np.float32), (128, 128))
    shared = dict(
        W1=np.asarray(W1, np.float32), W2=np.asarray(W2, np.float32),
        W3=np.asarray(W3, np.float32).reshape(128, 1),
        b1=np.asarray(b1, np.float32).reshape(128, 1),
        b2=np.asarray(b2, np.float32).reshape(128, 1),
        b3=np.broadcast_to(np.asarray(b3, np.float32).reshape(1, 1),
                           (128, 1)).astype(np.float32),
        iota=np.ascontiguousarray(iota.astype(BF16)),
    )
    in_maps = [{**shared, **pc} for pc in per_core]
    cfg = dict(N=N, NPC=NPC, B=B, NPAD=NPAD, G=G, GC=GC, RR=RR,
               T_br=[[int(x) for x in row] for row in T_br],
               boff=[int(o) for o in boff],
               pre_r=[[int(x) for x in row] for row in pre_r],
               chunks=chunks, call_w=call_w, icol=icol,
               W_chunk=W_chunk, Wmax=Wmax, ICtot=ICtot,
               total_cols=total_cols, Tmax=int(Tb.max()))
    return cfg, in_maps


# -------------------------------------------------------------- bass program
def _build(cfg):
    import concourse.bacc as bacc
    import concourse.mybir as mybir
    import concourse.tile as tile
    from concourse import bass

    dt = mybir.dt
    B = cfg["B"]
    NPAD = cfg["NPAD"]
    G = cfg["G"]
    GC = cfg["GC"]
    RR = cfg["RR"]
    T_br = cfg["T_br"]
    boff = cfg["boff"]
    pre_r = cfg["pre_r"]
    chunks = cfg["chunks"]
    call_w = cfg["call_w"]
    icol = cfg["icol"]
    Wmax = cfg["Wmax"]
    ICtot = cfg["ICtot"]
    total_cols = cfg["total_cols"]
    Tmax = cfg["Tmax"]
    rg = [list(range(NC))]

    nc = bacc.Bacc("TRN2", target_bir_lowering=False, num_devices=NC)

    def din(name, shape, dtype):
        return nc.dram_tensor(name, shape, dtype, kind="ExternalInput")

    hT_in = din("hT", [128, NPAD], dt.bfloat16)
    W1_in = din("W1", [128, 128], dt.float32)
    W2_in = din("W2", [128, 128], dt.float32)
    W3_in = din("W3", [128, 1], dt.float32)
    b1_in = din("b1", [128, 1], dt.float32)
    b2_in = din("b2", [128, 1], dt.float32)
    b3_in = din("b3", [128, 1], dt.float32)
    iota_in = din("iota", [128, 128], dt.bfloat16)
    idx16_in = din("idx16", [128, ICtot], dt.int16)
    dstloc_in = din("dstloc", [128, total_cols], dt.float32)
    gone_in = din("gonehot", [128, NPAD], dt.bfloat16)
    gidx_in = din("gidx", [128, 1], dt.int32)
    ns_in = din("ns", [128, B], dt.float32)
    ndbc_in = din("ndbc", [128, NPAD], dt.bfloat16)
    out_t = nc.dram_tensor("out", [G, 1], dt.float32, kind="ExternalOutput")

    x1_loc = nc.dram_tensor("x1_loc", [NPAD, 128], dt.bfloat16)
    x2_loc = nc.dram_tensor("x2_loc", [NPAD, 128], dt.bfloat16)
    x1_full = nc.dram_tensor("x1_full", [NPAD * NC, 128], dt.bfloat16,
                             addr_space="Shared")
    x2_full = nc.dram_tensor("x2_full", [NPAD * NC, 128], dt.bfloat16,
                             addr_space="Shared")
    pool_in = nc.dram_tensor("pool_in", [G + 128, 2], dt.float32)
    pool_out = nc.dram_tensor("pool_out", [G + 128, 2], dt.float32,
                              addr_space="Shared")

    with tile.TileContext(nc) as tc:
        with (
            tc.tile_pool(name="persist", bufs=1) as pp,
            tc.tile_pool(name="work", bufs=3) as wp,
            tc.tile_pool(name="gather", bufs=2) as gp,
            tc.tile_pool(name="onehot", bufs=3) as op_,
            tc.tile_pool(name="psA", bufs=2, space="PSUM") as psA,
            tc.tile_pool(name="psB", bufs=2, space="PSUM") as psB,
            tc.tile_pool(name="psC", bufs=1, space="PSUM") as psC,
        ):
            # ---------- constants / weights / metadata ----------
            iota = pp.tile([128, 128], dt.bfloat16)
            nc.sync.dma_start(out=iota[:], in_=iota_in[:])

            def load_w_bf(src_):
                f = wp.tile([128, 128], dt.float32, tag="wload")
                nc.sync.dma_start(out=f[:], in_=src_[:])
                bf = pp.tile([128, 128], dt.bfloat16, tag=src_.name + "bf")
                nc.vector.tensor_copy(out=bf[:], in_=f[:])
                return bf

            W1 = load_w_bf(W1_in)
            W2 = load_w_bf(W2_in)
            W3f = wp.tile([128, 1], dt.float32, tag="w3f")
            nc.sync.dma_start(out=W3f[:], in_=W3_in[:])
            W3 = pp.tile([128, 1], dt.bfloat16, tag="w3bf")
            nc.vector.tensor_copy(out=W3[:], in_=W3f[:])
            b1 = pp.tile([128, 1], dt.float32, tag="b1")
            nc.sync.dma_start(out=b1[:], in_=b1_in[:])
            b2 = pp.tile([128, 1], dt.float32, tag="b2")
            nc.sync.dma_start(out=b2[:], in_=b2_in[:])
            b3 = pp.tile([128, 1], dt.float32, tag="b3")
            nc.sync.dma_start(out=b3[:], in_=b3_in[:])

            norm_src = pp.tile([128, B], dt.float32, tag="ns")
            nc.sync.dma_start(out=norm_src[:], in_=ns_in[:])
            nd_bc = pp.tile([128, NPAD], dt.bfloat16, tag="ndbc")
            nc.sync.dma_start(out=nd_bc[:], in_=ndbc_in[:])

            idx16 = pp.tile([128, ICtot], dt.int16, tag="idx16")
            nc.sync.dma_start(out=idx16[:], in_=idx16_in[:])
            dstloc = pp.tile([128, total_cols], dt.float32, tag="dstloc")
            nc.sync.dma_start(out=dstloc[:], in_=dstloc_in[:])
            gone = pp.tile([128, NPAD], dt.bfloat16, tag="gone")
            nc.sync.dma_start(out=gone[:], in_=gone_in[:])
            gidx = pp.tile([128, 1], dt.int32, tag="gidx")
            nc.sync.dma_start(out=gidx[:], in_=gidx_in[:])

            # ---------- phase 1: project layer-1 for owned nodes ----------
            PCH = 8
            for c0 in range(0, B, PCH):
                nb = min(PCH, B - c0)
                hb = wp.tile([128, PCH * 128], dt.bfloat16, tag="hb")
                nc.sync.dma_start(out=hb[:, :nb * 128],
                                  in_=hT_in[:, c0 * 128:(c0 + nb) * 128])
                for i in range(nb):
                    b = c0 + i
                    ps = psA.tile([128, 128], dt.float32, tag="proj")
                    nc.tensor.matmul(out=ps[:], lhsT=hb[:, i * 128:(i + 1) * 128],
                                     rhs=W1[:], start=True, stop=True)
                    xsb = wp.tile([128, 128], dt.bfloat16, tag="xsb")
                    nc.vector.tensor_scalar(out=xsb[:], in0=ps[:],
                                            scalar1=norm_src[:, b:b + 1],
                                            scalar2=None,
                                            op0=mybir.AluOpType.mult)
                    nc.sync.dma_start(out=x1_loc[b * 128:(b + 1) * 128, :],
                                      in_=xsb[:])

            # ---------- all-gather x1 ----------
            nc.gpsimd.collective_compute(
                "AllGather", mybir.AluOpType.bypass, replica_groups=rg,
                ins=[x1_loc[:].opt()], outs=[x1_full[:].opt()])

            # ---------- shared per-layer machinery ----------
            def gather_chunk(ci, x_full):
                cw = sum(call_w[ci])
                xgc = gp.tile([128, Wmax], dt.bfloat16, tag="xgc")
                go = 0
                for r in range(RR):
                    w = call_w[ci][r]
                    if w == 0:
                        continue
                    lo = r * RSZ
                    hi = min(lo + RSZ, NPAD * NC)
                    nc.gpsimd.dma_gather(
                        out_ap=xgc[:, go:go + w].rearrange(
                            "p (j e) -> p j e", e=128),
                        in_ap=x_full[lo:hi, :],
                        idxs_ap=idx16[:, icol[ci][r]:icol[ci][r] + w // 16],
                        num_idxs=w,
                        num_idxs_reg=w,
                        elem_size=128,
                        single_packet=False,
                    )
                    go += w
                return xgc

            def block_agg(ci, b, xgc):
                # xg column offset of (b, r) inside the chunk tile
                Tb = sum(T_br[b])
                S = op_.tile([128, Tmax * 128], dt.bfloat16, tag="S")
                nc.vector.tensor_tensor(
                    out=S[:, :Tb * 128].rearrange("p (t j) -> p t j", j=128),
                    in0=iota[:].unsqueeze(1).to_broadcast([128, Tb, 128]),
                    in1=dstloc[:, boff[b]:boff[b] + Tb].unsqueeze(2)
                        .to_broadcast([128, Tb, 128]),
                    op=mybir.AluOpType.is_equal,
                )
                agg = psB.tile([128, 128], dt.float32, tag="agg")
                nmm = 0
                go = 0
                for r in range(RR):
                    xoff = go + sum(T_br[bb][r] for bb in chunks[ci]
                                    if bb < b) * 128
                    for t in range(T_br[b][r]):
                        scol = (pre_r[b][r] + t) * 128
                        nc.tensor.matmul(
                            out=agg[:],
                            lhsT=xgc[:, xoff + t * 128:xoff + (t + 1) * 128],
                            rhs=S[:, scol:scol + 128],
                            start=(nmm == 0), stop=(nmm == Tb - 1))
                        nmm += 1
                    go += call_w[ci][r]
                return agg

            def finish_h(b, agg, bias, out_ap):
                t1 = wp.tile([128, 128], dt.float32, tag="t1")
                nc.vector.tensor_tensor(out=t1[:], in0=agg[:],
                                        in1=nd_bc[:, b * 128:(b + 1) * 128],
                                        op=mybir.AluOpType.mult)
                nc.scalar.activation(out=out_ap, in_=t1[:],
                                     func=mybir.ActivationFunctionType.Relu,
                                     bias=bias[:, 0:1], scale=1.0)

            # ---------- layer 1 aggregate + layer 2 project ----------
            for ci in range(len(chunks)):
                xgc = gather_chunk(ci, x1_full)
                for b in chunks[ci]:
                    agg = block_agg(ci, b, xgc)
                    h1 = wp.tile([128, 128], dt.bfloat16, tag="h1")
                    finish_h(b, agg, b1, h1[:])
                    ps2 = psA.tile([128, 128], dt.float32, tag="proj")
                    nc.tensor.matmul(out=ps2[:], lhsT=h1[:],
                                     rhs=W2[:], start=True, stop=True)
                    x2sb = wp.tile([128, 128], dt.bfloat16, tag="xsb")
                    nc.vector.tensor_scalar(out=x2sb[:], in0=ps2[:],
                                            scalar1=norm_src[:, b:b + 1],
                                            scalar2=None,
                                            op0=mybir.AluOpType.mult)
                    nc.sync.dma_start(out=x2_loc[b * 128:(b + 1) * 128, :],
                                      in_=x2sb[:])

            # ---------- all-gather x2 ----------
            nc.gpsimd.collective_compute(
                "AllGather", mybir.AluOpType.bypass, replica_groups=rg,
                ins=[x2_loc[:].opt()], outs=[x2_full[:].opt()])

            # ---------- layer 2 aggregate + pooling ----------
            pool_acc = pp.tile([128, 2], dt.float32, tag="poolacc")
            nc.gpsimd.memset(pool_acc[:], 0.0)
            ones_col = pp.tile([128, 1], dt.bfloat16, tag="ones")
            nc.gpsimd.memset(ones_col[:], 1.0)

            for ci in range(len(chunks)):
                xgc = gather_chunk(ci, x2_full)
                for b in chunks[ci]:
                    agg = block_agg(ci, b, xgc)
                    h2 = wp.tile([128, 128], dt.bfloat16, tag="h2")
                    finish_h(b, agg, b2, h2[:])
                    psd = psC.tile([128, 1], dt.float32, tag="dots")
                    nc.tensor.matmul(out=psd[:], lhsT=h2[:], rhs=W3[:],
                                     start=True, stop=True)
                    rhs2 = wp.tile([128, 2], dt.bfloat16, tag="rhs2")
                    nc.vector.tensor_copy(out=rhs2[:, 0:1], in_=psd[:])
                    nc.vector.tensor_copy(out=rhs2[:, 1:2], in_=ones_col[:])
                    psp = psC.tile([128, 2], dt.float32, tag="poolmm")
                    nc.tensor.matmul(out=psp[:],
                                     lhsT=gone[:, b * 128:(b + 1) * 128],
                                     rhs=rhs2[:], start=True, stop=True)
                    nc.vector.tensor_add(out=pool_acc[:], in0=pool_acc[:],
                                         in1=psp[:])

            # ---------- combine pools across cores ----------
            zt = wp.tile([128, 2 * (GC + 1)], dt.float32, tag="zt")
            nc.gpsimd.memset(zt[:], 0.0)
            nc.sync.dma_start(
                out=bass.AP(pool_in, 0, [[2, 128], [256, GC + 1], [1, 2]]),
                in_=zt[:].rearrange("p (g c) -> p g c", c=2))
            nc.gpsimd.indirect_dma_start(
                out=pool_in[:],
                out_offset=bass.IndirectOffsetOnAxis(ap=gidx[:, 0:1], axis=0),
                in_=pool_acc[:], in_offset=None)
            nc.gpsimd.collective_compute(
                "AllReduce", mybir.AluOpType.add, replica_groups=rg,
                ins=[pool_in[:].opt()], outs=[pool_out[:].opt()])

            # ---------- finish: out = dot/cnt + b3 ----------
            dots_t = wp.tile([128, GC], dt.float32, tag="dotst")
            cnt = wp.tile([128, GC], dt.float32, tag="cnt")
            nc.sync.dma_start(
                out=dots_t[:], in_=bass.AP(pool_out, 0, [[2, 128], [256, GC]]))
            nc.sync.dma_start(
                out=cnt[:], in_=bass.AP(pool_out, 1, [[2, 128], [256, GC]]))
            nc.vector.tensor_scalar_max(out=cnt[:], in0=cnt[:], scalar1=1.0)
            rec = wp.tile([128, GC], dt.float32, tag="recc")
            nc.vector.reciprocal(out=rec[:], in_=cnt[:])
            res = wp.tile([128, GC], dt.float32, tag="res")
            nc.vector.tensor_tensor(out=res[:], in0=dots_t[:], in1=rec[:],
                                    op=mybir.AluOpType.mult)
            nc.vector.tensor_scalar(out=res[:], in0=res[:], scalar1=b3[:, 0:1],
                                    scalar2=None, op0=mybir.AluOpType.add)
            nc.sync.dma_start(
                out=bass.AP(out_t, 0, [[1, 128], [128, GC]]), in_=res[:])

    nc.compile()
    return nc


_CACHE = {}


def _get_nc(cfg):
    key = (cfg["N"], cfg["G"], cfg["total_cols"], cfg["ICtot"])
    if key not in _CACHE:
        _CACHE[key] = _build(cfg)
    return _CACHE[key]


def kernel(**inputs) -> np.ndarray:
    from concourse import bass_utils

    cfg, in_maps = _prep(**inputs)
    nc = _get_nc(cfg)
    res = bass_utils.run_bass_kernel_spmd(
        nc, in_maps, core_ids=list(range(NC)), trace=False)
    return np.asarray(res.results[0]["out"], dtype=np.float32)
